# revision 22
# baseline (speedup 1.0000x reference)
"""GAU (gated attention unit) Trainium2 Bass kernel.

Sharding: 8 cores = 4 batches x 2 E-halves.
  core c -> batch b = c//2, E-half h = c%2 (cols h*768:(h+1)*768 of E=1536).
Each core computes, for its batch:
  LN stats, z/q/k (S=128, shared across E), u/v for its E-half,
  qk^T = k @ q^T, a^T = relu(qk^T)^2 (mask folded into gamma_k/beta_k),
  attn^T = v^T @ a^T, g = u^T * attn^T, out_partial = g^T @ Wo_half.
Host: out[b] = part[2b] + part[2b+1] + bo + x[b].

Precision: fp32 data everywhere; matmuls run as float32r (full PE rate at
moving-dim >= 256). a^T and v are stored bf16 (their rounding errors average
out over the 2048-key contraction).

LayerNorm without transposes: stats via bn_stats on token-major x tiles;
y = xT * rstd (broadcast tile); the mean term is folded into every matmul as
a rank-2 PSUM correction with lhsT/rhs rows ([-colsum(W); bias] x [m2; ones]),
m2 = mu * rstd.
"""

import numpy as np
from contextlib import ExitStack

import concourse.bass as bass
import concourse.tile as tile
from concourse import bacc, mybir
from concourse.bass_utils import run_bass_kernel_spmd
from concourse.masks import make_identity

# Problem dims (hardcoded per the task contract)
B, T, D, S, E = 4, 2048, 768, 128, 1536
EH = E // 2          # per-core E half
P = 128
ND = D // P          # 6 d-chunks
NE = EH // P         # 6 e-chunks
NT = T // P          # 16 token chunks
TT = 512             # t-tile (phases B1/B2) and attention t-block
NTT = T // TT        # 4
LN_EPS = 1e-5

F32 = mybir.dt.float32
F32R = mybir.dt.float32r
BF16 = mybir.dt.bfloat16
AF = mybir.ActivationFunctionType
ALU = mybir.AluOpType

N_CORES = 8


def _r(ap):
    """View an fp32 AP as float32r for full-rate PE matmuls."""
    return ap.bitcast(F32R)


def build_module():
    nc = bacc.Bacc("TRN2", debug=False, num_devices=N_CORES, num_swdge_queues=4)

    # ---- DRAM I/O ----
    xT_d = nc.dram_tensor("xT", [D, T], F32, kind="ExternalInput").ap()
    x_d = nc.dram_tensor("x", [T, D], F32, kind="ExternalInput").ap()
    gq_d = nc.dram_tensor("gqT", [S, T], F32, kind="ExternalInput").ap()
    bq_d = nc.dram_tensor("bqT", [S, T], F32, kind="ExternalInput").ap()
    gk_d = nc.dram_tensor("gkT", [S, T], F32, kind="ExternalInput").ap()
    bk_d = nc.dram_tensor("bkT", [S, T], F32, kind="ExternalInput").ap()
    wz_d = nc.dram_tensor("Wz", [D, S], F32, kind="ExternalInput").ap()
    wu_d = nc.dram_tensor("Wu", [D, EH], F32, kind="ExternalInput").ap()
    wv_d = nc.dram_tensor("Wv", [D, EH], F32, kind="ExternalInput").ap()
    wo_d = nc.dram_tensor("Wo", [EH, D], F32, kind="ExternalInput").ap()
    cz_d = nc.dram_tensor("Cz", [2, S], F32, kind="ExternalInput").ap()
    cu_d = nc.dram_tensor("Cu", [2, EH], F32, kind="ExternalInput").ap()
    cv_d = nc.dram_tensor("Cv", [2, EH], F32, kind="ExternalInput").ap()
    out_d = nc.dram_tensor("outp", [T, D], F32, kind="ExternalOutput").ap()
    # scratch row for the rstd partition-broadcast bounce
    scr_d = nc.dram_tensor("rstd_scr", [1, T], F32, kind="Internal").ap()

    xT_r = xT_d.rearrange("(c p) t -> p c t", p=P)   # [128, 6, 2048]
    wu_r = wu_d.rearrange("(c p) e -> p c e", p=P)   # [128, 6, 768]
    wv_r = wv_d.rearrange("(c p) e -> p c e", p=P)
    wz_r = wz_d.rearrange("(c p) s -> p c s", p=P)   # [128, 6, 128]
    wo_r = wo_d.rearrange("(c p) d -> p c d", p=P)   # [128, 6, 768]

    with tile.TileContext(nc) as tc, ExitStack() as ctx:
        # ---------- persistent pools ----------
        persist = ctx.enter_context(tc.tile_pool(name="persist", bufs=1))
        ident = persist.tile([P, P], F32)
        make_identity(nc, ident)
        eps_t = persist.tile([P, 1], F32)
        nc.vector.memset(eps_t, LN_EPS)
        # S2 rows: 0 = ones, 1 = m2 = mu*rstd, 2 = rstd.
        # Rank-2 correction rhs is S2[0:2] (base partition 0); rstd (row 2)
        # is only touched by DMA, which has no base-partition restriction.
        # fp32r matmul operands must be written by a rounding instruction,
        # so every tile feeding a matmul is declared float32r and produced
        # by a DVE/ACT op (DMA loads go through a .bitcast(F32) view).
        S3 = persist.tile([3, T], F32R)
        S2 = S3[0:2, :]
        rstd_b = persist.tile([P, T], F32)           # rstd broadcast to 128 parts
        qT = persist.tile([S, T], F32R)
        kT = persist.tile([S, T], F32R)

        uT_pool = ctx.enter_context(tc.tile_pool(name="uT", bufs=1))
        uT = uT_pool.tile([P, NE, T], F32R)          # 48KB/part

        # ---------- phase A+B1: stats, y, z/q/k, u ----------
        with (
            tc.tile_pool(name="statw", bufs=3) as sw,
            tc.tile_pool(name="statp", bufs=2, space="PSUM") as sp,
            tc.tile_pool(name="w1", bufs=1) as w1,
            tc.tile_pool(name="b1w", bufs=2) as b1w,
            tc.tile_pool(name="b1p", bufs=4, space="PSUM") as b1p,
        ):
            def load_f32r_w(pool, dst, src_r, nchunk):
                """DMA f32 chunks into a staging tile, round into dst (f32r)."""
                for c in range(nchunk):
                    stg = pool.tile([P, dst.shape[-1]], F32, tag="stage")
                    nc.gpsimd.dma_start(out=stg, in_=src_r[:, c, :])
                    nc.scalar.copy(out=dst[:, c, :], in_=stg)

            def load_f32r_c(pool, dst, src):
                stg = pool.tile(list(dst.shape), F32, tag="cstage")
                nc.gpsimd.dma_start(out=stg, in_=src)
                nc.vector.tensor_copy(out=dst, in_=stg)

            wz_t = w1.tile([P, ND, S], F32R)
            load_f32r_w(b1w, wz_t, wz_r, ND)
            wu_t = w1.tile([P, ND, EH], F32R)
            load_f32r_w(b1w, wu_t, wu_r, ND)
            cz_t = w1.tile([2, S], F32R)
            load_f32r_c(b1w, cz_t, cz_d)
            cu_t = w1.tile([2, EH], F32R)
            load_f32r_c(b1w, cu_t, cu_d)

            for tt in range(NTT):
                ts_ = slice(tt * TT, (tt + 1) * TT)
                # --- LN stats for the 4 token chunks of this t-tile ---
                for sub in range(TT // P):
                    it = tt * (TT // P) + sub
                    xt = sw.tile([P, D], F32, tag="xtile")
                    nc.gpsimd.dma_start(out=xt, in_=x_d[it * P:(it + 1) * P, :])
                    st = sw.tile([P, 3, 6], F32, tag="bnst")
                    for g in range(3):
                        nc.vector.bn_stats(
                            out=st[:, g, :], in_=xt[:, g * 256:(g + 1) * 256]
                        )
                    mv = sw.tile([P, 2], F32, tag="mv")
                    nc.vector.bn_aggr(out=mv, in_=st)
                    # pair cols: 0 = ones, 1 = m2 = mu*rstd, 2 = rstd
                    pair = sw.tile([P, 3], F32, tag="pair")
                    nc.vector.memset(pair[:, 0:1], 1.0)
                    nc.scalar.activation(
                        out=pair[:, 2:3], in_=mv[:, 1:2], func=AF.Sqrt,
                        bias=eps_t, scale=1.0,
                    )
                    nc.vector.reciprocal(out=pair[:, 2:3], in_=pair[:, 2:3])
                    nc.vector.tensor_mul(
                        out=pair[:, 1:2], in0=mv[:, 0:1], in1=pair[:, 2:3]
                    )
                    pt = sp.tile([3, P], F32, tag="pt")
                    nc.tensor.transpose(pt, pair, ident)
                    nc.vector.tensor_copy(
                        out=S3[:, it * P:(it + 1) * P], in_=pt
                    )
                # broadcast rstd for this t-tile via DRAM bounce
                nc.gpsimd.dma_start(
                    out=scr_d[:, ts_], in_=S3[2:3, ts_].bitcast(F32)
                )
                bcast_src = bass.AP(
                    tensor=scr_d.tensor, offset=scr_d.offset + tt * TT,
                    ap=[[0, P], [1, TT]],
                )
                nc.gpsimd.dma_start(out=rstd_b[:, ts_], in_=bcast_src)

                # --- y block: xT * rstd (rounded to f32r on write) ---
                yb_raw = b1w.tile([P, ND, TT], F32, tag="yb_raw")
                nc.gpsimd.dma_start(out=yb_raw, in_=xT_r[:, :, ts_])
                yb = b1w.tile([P, ND, TT], F32R, tag="yb")
                for c in range(ND):
                    nc.vector.tensor_mul(
                        out=yb[:, c, :], in0=yb_raw[:, c, :],
                        in1=rstd_b[:, ts_],
                    )

                # --- z -> q,k ---
                zp = b1p.tile([S, TT], F32, tag="mm")
                for c in range(ND):
                    nc.tensor.matmul(
                        zp, wz_t[:, c, :], yb[:, c, :],
                        start=(c == 0), stop=False,
                    )
                nc.tensor.matmul(zp, cz_t, S2[:, ts_],
                                 start=False, stop=True)
                gq = b1w.tile([S, TT], F32, tag="gq")
                nc.gpsimd.dma_start(out=gq, in_=gq_d[:, ts_])
                bq = b1w.tile([S, TT], F32, tag="bq")
                nc.gpsimd.dma_start(out=bq, in_=bq_d[:, ts_])
                gk = b1w.tile([S, TT], F32, tag="gk")
                nc.gpsimd.dma_start(out=gk, in_=gk_d[:, ts_])
                bk = b1w.tile([S, TT], F32, tag="bk")
                nc.gpsimd.dma_start(out=bk, in_=bk_d[:, ts_])
                nc.vector.tensor_mul(out=qT[:, ts_], in0=zp, in1=gq)
                nc.vector.tensor_add(out=qT[:, ts_], in0=qT[:, ts_], in1=bq)
                nc.vector.tensor_mul(out=kT[:, ts_], in0=zp, in1=gk)
                nc.vector.tensor_add(out=kT[:, ts_], in0=kT[:, ts_], in1=bk)

                # --- u ---
                for e in range(NE):
                    up = b1p.tile([P, TT], F32, tag="mm")
                    for c in range(ND):
                        nc.tensor.matmul(
                            up, wu_t[:, c, e * P:(e + 1) * P],
                            yb[:, c, :], start=(c == 0), stop=False,
                        )
                    nc.tensor.matmul(
                        up, cu_t[:, e * P:(e + 1) * P], S2[:, ts_],
                        start=False, stop=True,
                    )
                    nc.scalar.copy(out=uT[:, e, ts_], in_=up)

        # ---------- phase B2: v (token-major, bf16) ----------
        v_pool = ctx.enter_context(tc.tile_pool(name="v", bufs=1))
        v_t = v_pool.tile([P, NT, EH], BF16)         # 24KB/part
        with (
            tc.tile_pool(name="w2", bufs=1) as w2,
            tc.tile_pool(name="b2w", bufs=2) as b2w,
            tc.tile_pool(name="b2p", bufs=4, space="PSUM") as b2p,
        ):
            wv_t = w2.tile([P, ND, EH], F32R)
            for c in range(ND):
                stg = b2w.tile([P, EH], F32, tag="stage2")
                nc.gpsimd.dma_start(out=stg, in_=wv_r[:, c, :])
                nc.scalar.copy(out=wv_t[:, c, :], in_=stg)
            cv_t = w2.tile([2, EH], F32R)
            cstg = b2w.tile([2, EH], F32, tag="cstage2")
            nc.gpsimd.dma_start(out=cstg, in_=cv_d)
            nc.vector.tensor_copy(out=cv_t, in_=cstg)
            for tt in range(NTT):
                ts_ = slice(tt * TT, (tt + 1) * TT)
                yb_raw = b2w.tile([P, ND, TT], F32, tag="yb2_raw")
                nc.gpsimd.dma_start(out=yb_raw, in_=xT_r[:, :, ts_])
                yb = b2w.tile([P, ND, TT], F32R, tag="yb2")
                for c in range(ND):
                    nc.vector.tensor_mul(
                        out=yb[:, c, :], in0=yb_raw[:, c, :],
                        in1=rstd_b[:, ts_],
                    )
                for tch in range(TT // P):
                    it = tt * (TT // P) + tch
                    tc_ = slice(it * P, (it + 1) * P)
                    for (e0, ew) in ((0, 384), (384, 384)):
                        vp = b2p.tile([P, 384], F32, tag="mm")
                        for c in range(ND):
                            nc.tensor.matmul(
                                vp, yb[:, c, tch * P:(tch + 1) * P],
                                wv_t[:, c, e0:e0 + ew],
                                start=(c == 0), stop=False,
                            )
                        nc.tensor.matmul(
                            vp, S2[:, tc_], cv_t[:, e0:e0 + ew],
                            start=False, stop=True,
                        )
                        nc.scalar.copy(out=v_t[:, it, e0:e0 + ew], in_=vp)

        # ---------- phase C: attention + output ----------
        with (
            tc.tile_pool(name="w3", bufs=1) as w3,
            tc.tile_pool(name="atp", bufs=2) as atp,
            tc.tile_pool(name="c3w", bufs=3) as c3w,
            tc.tile_pool(name="qkp", bufs=2, space="PSUM") as qkp,
            tc.tile_pool(name="attp", bufs=2, space="PSUM") as attp,
            tc.tile_pool(name="outp_p", bufs=2, space="PSUM") as outp_p,
        ):
            wo_t = w3.tile([P, NE, D], F32R)
            for c in range(NE):
                stg = c3w.tile([P, D], F32, tag="stage3")
                nc.gpsimd.dma_start(out=stg, in_=wo_r[:, c, :])
                nc.scalar.copy(out=wo_t[:, c, :], in_=stg)
            for tb in range(NTT):
                tbs = slice(tb * TT, (tb + 1) * TT)
                aT = atp.tile([P, NT, TT], BF16, tag="aT")
                for uc in range(NT):
                    qk = qkp.tile([P, TT], F32, tag="qk")
                    nc.tensor.matmul(
                        qk, kT[:, uc * P:(uc + 1) * P], qT[:, tbs],
                        start=True, stop=True,
                    )
                    rt = c3w.tile([P, TT], F32, tag="rt")
                    nc.vector.tensor_scalar_max(out=rt, in0=qk, scalar1=0.0)
                    nc.scalar.square(out=aT[:, uc, :], in_=rt)
                for e in range(NE):
                    at_ps = attp.tile([P, TT], F32, tag="att")
                    for uc in range(NT):
                        nc.tensor.matmul(
                            at_ps, v_t[:, uc, e * P:(e + 1) * P],
                            aT[:, uc, :],
                            start=(uc == 0), stop=(uc == NT - 1),
                        )
                    # g = u * attn, in place over uT
                    nc.vector.tensor_mul(
                        out=uT[:, e, tbs], in0=at_ps, in1=uT[:, e, tbs]
                    )
                for tch in range(TT // P):
                    it = tb * (TT // P) + tch
                    tc_ = slice(it * P, (it + 1) * P)
                    osb = c3w.tile([P, D], F32, tag="osb")
                    for (d0, dw) in ((0, 384), (384, 384)):
                        op_ = outp_p.tile([P, 384], F32, tag="mo")
                        for e in range(NE):
                            nc.tensor.matmul(
                                op_, _r(uT[:, e, tc_]),
                                _r(wo_t[:, e, d0:d0 + dw]),
                                start=(e == 0), stop=(e == NE - 1),
                            )
                        nc.scalar.copy(out=osb[:, d0:d0 + dw], in_=op_)
                    nc.gpsimd.dma_start(out=out_d[tc_, :], in_=osb)

    nc.finalize()
    return nc


def prep_core_inputs(inputs):
    """Host-side slicing: returns the list of 8 per-core input maps."""
    f = np.float32
    x = np.asarray(inputs["x"], f)
    mask = np.asarray(inputs["mask"])
    ln_w = np.asarray(inputs["ln_w"], f)
    ln_b = np.asarray(inputs["ln_b"], f)
    Wz = np.asarray(inputs["Wz"], f)
    bz = np.asarray(inputs["bz"], f)
    Wu = np.asarray(inputs["Wu"], f)
    bu = np.asarray(inputs["bu"], f)
    Wv = np.asarray(inputs["Wv"], f)
    bv = np.asarray(inputs["bv"], f)
    Wo = np.asarray(inputs["Wo"], f)
    gq = np.asarray(inputs["gamma_q"], f)
    bq = np.asarray(inputs["beta_q"], f)
    gk = np.asarray(inputs["gamma_k"], f)
    bk = np.asarray(inputs["beta_k"], f)

    # fold ln_w into the weights, ln_b into the matmul biases
    Wz_e = np.ascontiguousarray(ln_w[:, None] * Wz)
    Wu_e = ln_w[:, None] * Wu
    Wv_e = ln_w[:, None] * Wv
    bz_e = ln_b @ Wz + bz
    bu_e = ln_b @ Wu + bu
    bv_e = ln_b @ Wv + bv

    gqT = np.ascontiguousarray(gq.T)
    bqT = np.ascontiguousarray(bq.T)
    gkT = np.ascontiguousarray(gk.T)
    bkT = np.ascontiguousarray(bk.T)
    # C rows pair with S2 rows: row0 <-> ones (bias), row1 <-> m2 (-colsum)
    Cz = np.ascontiguousarray(np.stack([bz_e, -Wz_e.sum(0)]).astype(f))

    in_maps = []
    for c in range(N_CORES):
        b, h = c // 2, c % 2
        cols = slice(h * EH, (h + 1) * EH)
        keep = (~mask[b]).astype(f)  # 1 = attend, 0 = masked-out key
        Wu_h = np.ascontiguousarray(Wu_e[:, cols])
        Wv_h = np.ascontiguousarray(Wv_e[:, cols])
        in_maps.append({
            "x": np.ascontiguousarray(x[b]),
            "xT": np.ascontiguousarray(x[b].T),
            "gqT": gqT,
            "bqT": bqT,
            "gkT": np.ascontiguousarray(gkT * keep[None, :]),
            "bkT": np.ascontiguousarray(bkT * keep[None, :]),
            "Wz": Wz_e,
            "Wu": Wu_h,
            "Wv": Wv_h,
            "Wo": np.ascontiguousarray(Wo[cols, :]),
            "Cz": Cz,
            "Cu": np.ascontiguousarray(
                np.stack([bu_e[cols], -Wu_h.sum(0)]).astype(f)),
            "Cv": np.ascontiguousarray(
                np.stack([bv_e[cols], -Wv_h.sum(0)]).astype(f)),
        })
    return in_maps


def combine_outputs(inputs, parts):
    """parts: list of 8 [T, D] partial outputs -> full [B, T, D]."""
    f = np.float32
    x = np.asarray(inputs["x"], f)
    bo = np.asarray(inputs["bo"], f)
    out = np.empty((B, T, D), f)
    for b in range(B):
        out[b] = parts[2 * b] + parts[2 * b + 1] + bo[None, :] + x[b]
    return out


_NC_CACHE = None


def run(inputs, trace=False, **kw):
    global _NC_CACHE
    if _NC_CACHE is None:
        _NC_CACHE = build_module()
    nc = _NC_CACHE
    in_maps = prep_core_inputs(inputs)
    res = run_bass_kernel_spmd(
        nc, in_maps, core_ids=list(range(N_CORES)), trace=trace, **kw
    )
    parts = [r["outp"] for r in res.results]
    return combine_outputs(inputs, parts), res


def kernel(**inputs):
    out, _ = run(inputs)
    return out


# revision 24
# speedup vs baseline: 1.3435x; 1.3435x over previous
"""GAU (gated attention unit) Trainium2 Bass kernel.

Sharding: 8 cores = 4 batches x 2 E-halves.
  core c -> batch b = c//2, E-half h = c%2 (cols h*768:(h+1)*768 of E=1536).
Each core computes, for its batch:
  LN stats, z/q/k (S=128, shared across E), u/v for its E-half,
  qk^T = k @ q^T, a^T = relu(qk^T)^2 (mask folded into gamma_k/beta_k),
  attn^T = v^T @ a^T, g = u^T * attn^T, out_partial = g^T @ Wo_half.
Host: out[b] = part[2b] + part[2b+1] + bo + x[b].

Precision: all matmuls use fp16 operands with fp32 PSUM accumulation (fp16
streams 1 cycle/row on the PE and enables fast weight loads; fp32/fp32r
stream at 2-4 cycles/row with slow unhidden LDWEIGHTS). Measured end-to-end
relative error ~6e-3 vs the fp32 reference, dominated by fp16 rounding that
averages out over the 768/2048-deep contractions. The q/k affine params and
LN statistics are computed in fp32.

LayerNorm without transposes: stats via bn_stats on token-major x tiles;
y = xT * rstd; the mean term is folded into every matmul as a rank-2 PSUM
correction with rows ([bias; -colsum(W)] x [ones; m2]), m2 = mu * rstd.
"""

import numpy as np
import ml_dtypes
from contextlib import ExitStack

import concourse.bass as bass
import concourse.tile as tile
from concourse import bacc, mybir
from concourse.bass_utils import run_bass_kernel_spmd
from concourse.masks import make_identity

# Problem dims (hardcoded per the task contract)
B, T, D, S, E = 4, 2048, 768, 128, 1536
EH = E // 2          # per-core E half
P = 128
ND = D // P          # 6 d-chunks
NE = EH // P         # 6 e-chunks
NT = T // P          # 16 token chunks
TT = 512             # t-tile (phase B) and attention t-block
NTT = T // TT        # 4
LN_EPS = 1e-5

F32 = mybir.dt.float32
FP16 = mybir.dt.float16
AF = mybir.ActivationFunctionType
ALU = mybir.AluOpType
NPFP16 = np.float16

N_CORES = 8


def build_module():
    nc = bacc.Bacc("TRN2", debug=False, num_devices=N_CORES, num_swdge_queues=4)

    # ---- DRAM I/O ----
    xT_d = nc.dram_tensor("xT", [D, T], FP16, kind="ExternalInput").ap()
    x_d = nc.dram_tensor("x", [T, D], F32, kind="ExternalInput").ap()
    gq_d = nc.dram_tensor("gqT", [S, T], F32, kind="ExternalInput").ap()
    bq_d = nc.dram_tensor("bqT", [S, T], F32, kind="ExternalInput").ap()
    gk_d = nc.dram_tensor("gkT", [S, T], F32, kind="ExternalInput").ap()
    bk_d = nc.dram_tensor("bkT", [S, T], F32, kind="ExternalInput").ap()
    wz_d = nc.dram_tensor("Wz", [D, S], FP16, kind="ExternalInput").ap()
    wu_d = nc.dram_tensor("Wu", [D, EH], FP16, kind="ExternalInput").ap()
    wv_d = nc.dram_tensor("Wv", [D, EH], FP16, kind="ExternalInput").ap()
    wo_d = nc.dram_tensor("Wo", [EH, D], FP16, kind="ExternalInput").ap()
    cz_d = nc.dram_tensor("Cz", [2, S], FP16, kind="ExternalInput").ap()
    cu_d = nc.dram_tensor("Cu", [2, EH], FP16, kind="ExternalInput").ap()
    cv_d = nc.dram_tensor("Cv", [2, EH], FP16, kind="ExternalInput").ap()
    out_d = nc.dram_tensor("outp", [T, D], F32, kind="ExternalOutput").ap()
    # scratch row for the rstd partition-broadcast bounce
    scr_d = nc.dram_tensor("rstd_scr", [1, T], F32, kind="Internal").ap()

    xT_r = xT_d.rearrange("(c p) t -> p c t", p=P)   # [128, 6, 2048]
    wu_r = wu_d.rearrange("(c p) e -> p c e", p=P)   # [128, 6, 768]
    wv_r = wv_d.rearrange("(c p) e -> p c e", p=P)
    wz_r = wz_d.rearrange("(c p) s -> p c s", p=P)   # [128, 6, 128]
    wo_r = wo_d.rearrange("(c p) d -> p c d", p=P)   # [128, 6, 768]

    with tile.TileContext(nc) as tc, ExitStack() as ctx:
        # ---------- persistent pools ----------
        persist = ctx.enter_context(tc.tile_pool(name="persist", bufs=1))
        ident = persist.tile([P, P], F32)
        make_identity(nc, ident)
        eps_t = persist.tile([P, 1], F32)
        nc.vector.memset(eps_t, LN_EPS)
        # S3 rows (fp32): 0 = ones, 1 = m2 = mu*rstd, 2 = rstd.
        # S2b is the fp16 copy of rows 0:2 used as the rank-2 matmul operand.
        S3 = persist.tile([3, T], F32)
        S2b = persist.tile([2, T], FP16)
        rstd_b = persist.tile([P, T], F32)           # rstd broadcast to 128 parts
        qT = persist.tile([S, T], FP16)
        kT = persist.tile([S, T], FP16)
        uT = persist.tile([P, NE, T], FP16)          # 24KB/part
        v_t = persist.tile([P, NT, EH], FP16)        # 24KB/part

        # ---------- phase B: stats + z/q/k + u + v, per t-tile ----------
        with (
            tc.tile_pool(name="statw", bufs=3) as sw,
            tc.tile_pool(name="statp", bufs=2, space="PSUM") as sp,
            tc.tile_pool(name="w1", bufs=1) as w1,
            tc.tile_pool(name="b1w", bufs=2) as b1w,
            tc.tile_pool(name="b1p", bufs=4, space="PSUM") as b1p,
        ):
            wz_t = w1.tile([P, ND, S], FP16)
            nc.sync.dma_start(out=wz_t, in_=wz_r)
            wu_t = w1.tile([P, ND, EH], FP16)
            nc.sync.dma_start(out=wu_t, in_=wu_r)
            wv_t = w1.tile([P, ND, EH], FP16)
            nc.sync.dma_start(out=wv_t, in_=wv_r)
            cz_t = w1.tile([2, S], FP16)
            nc.sync.dma_start(out=cz_t, in_=cz_d)
            cu_t = w1.tile([2, EH], FP16)
            nc.sync.dma_start(out=cu_t, in_=cu_d)
            cv_t = w1.tile([2, EH], FP16)
            nc.sync.dma_start(out=cv_t, in_=cv_d)

            for tt in range(NTT):
                ts_ = slice(tt * TT, (tt + 1) * TT)
                # --- LN stats for the 4 token chunks of this t-tile ---
                for sub in range(TT // P):
                    it = tt * (TT // P) + sub
                    xt = sw.tile([P, D], F32, tag="xtile")
                    nc.sync.dma_start(out=xt, in_=x_d[it * P:(it + 1) * P, :])
                    st = sw.tile([P, 3, 6], F32, tag="bnst")
                    for g in range(3):
                        nc.vector.bn_stats(
                            out=st[:, g, :], in_=xt[:, g * 256:(g + 1) * 256]
                        )
                    mv = sw.tile([P, 2], F32, tag="mv")
                    nc.vector.bn_aggr(out=mv, in_=st)
                    # pair cols: 0 = ones, 1 = m2 = mu*rstd, 2 = rstd
                    pair = sw.tile([P, 3], F32, tag="pair")
                    nc.vector.memset(pair[:, 0:1], 1.0)
                    nc.scalar.activation(
                        out=pair[:, 2:3], in_=mv[:, 1:2], func=AF.Sqrt,
                        bias=eps_t, scale=1.0,
                    )
                    nc.vector.reciprocal(out=pair[:, 2:3], in_=pair[:, 2:3])
                    nc.vector.tensor_mul(
                        out=pair[:, 1:2], in0=mv[:, 0:1], in1=pair[:, 2:3]
                    )
                    pt = sp.tile([3, P], F32, tag="pt")
                    nc.tensor.transpose(pt, pair, ident)
                    nc.vector.tensor_copy(
                        out=S3[:, it * P:(it + 1) * P], in_=pt
                    )
                # fp16 rank-2 operand rows [ones; m2]
                nc.vector.tensor_copy(out=S2b[:, ts_], in_=S3[0:2, ts_])
                # broadcast rstd for this t-tile via DRAM bounce
                nc.sync.dma_start(out=scr_d[:, ts_], in_=S3[2:3, ts_])
                bcast_src = bass.AP(
                    tensor=scr_d.tensor, offset=scr_d.offset + tt * TT,
                    ap=[[0, P], [1, TT]],
                )
                nc.sync.dma_start(out=rstd_b[:, ts_], in_=bcast_src)

                # --- y block: xT * rstd (fp16) ---
                xb = b1w.tile([P, ND, TT], FP16, tag="xb")
                nc.sync.dma_start(out=xb, in_=xT_r[:, :, ts_])
                yb = b1w.tile([P, ND, TT], FP16, tag="yb")
                for c in range(ND):
                    nc.vector.tensor_mul(
                        out=yb[:, c, :], in0=xb[:, c, :], in1=rstd_b[:, ts_]
                    )

                # --- z -> q,k ---
                zp = b1p.tile([S, TT], F32, tag="mm")
                for c in range(ND):
                    nc.tensor.matmul(
                        zp, wz_t[:, c, :], yb[:, c, :],
                        start=(c == 0), stop=False,
                    )
                nc.tensor.matmul(zp, cz_t, S2b[:, ts_], start=False, stop=True)
                gq = b1w.tile([S, TT], F32, tag="gq")
                nc.sync.dma_start(out=gq, in_=gq_d[:, ts_])
                bq = b1w.tile([S, TT], F32, tag="bq")
                nc.sync.dma_start(out=bq, in_=bq_d[:, ts_])
                gk = b1w.tile([S, TT], F32, tag="gk")
                nc.sync.dma_start(out=gk, in_=gk_d[:, ts_])
                bk = b1w.tile([S, TT], F32, tag="bk")
                nc.sync.dma_start(out=bk, in_=bk_d[:, ts_])
                qf = b1w.tile([S, TT], F32, tag="qf")
                nc.vector.tensor_mul(out=qf, in0=zp, in1=gq)
                nc.vector.tensor_add(out=qT[:, ts_], in0=qf, in1=bq)
                kf = b1w.tile([S, TT], F32, tag="kf")
                nc.vector.tensor_mul(out=kf, in0=zp, in1=gk)
                nc.vector.tensor_add(out=kT[:, ts_], in0=kf, in1=bk)

                # --- u (E-major) ---
                for e in range(NE):
                    up = b1p.tile([P, TT], F32, tag="mm")
                    for c in range(ND):
                        nc.tensor.matmul(
                            up, wu_t[:, c, e * P:(e + 1) * P], yb[:, c, :],
                            start=(c == 0), stop=False,
                        )
                    nc.tensor.matmul(
                        up, cu_t[:, e * P:(e + 1) * P], S2b[:, ts_],
                        start=False, stop=True,
                    )
                    nc.scalar.copy(out=uT[:, e, ts_], in_=up)

                # --- v (token-major) ---
                for tch in range(TT // P):
                    it = tt * (TT // P) + tch
                    tc_ = slice(it * P, (it + 1) * P)
                    for (e0, ew) in ((0, 384), (384, 384)):
                        vp = b1p.tile([P, 384], F32, tag="mm")
                        for c in range(ND):
                            nc.tensor.matmul(
                                vp, yb[:, c, tch * P:(tch + 1) * P],
                                wv_t[:, c, e0:e0 + ew],
                                start=(c == 0), stop=False,
                            )
                        nc.tensor.matmul(
                            vp, S2b[:, tc_], cv_t[:, e0:e0 + ew],
                            start=False, stop=True,
                        )
                        nc.scalar.copy(out=v_t[:, it, e0:e0 + ew], in_=vp)

        # ---------- phase C: attention + output ----------
        with (
            tc.tile_pool(name="w3", bufs=1) as w3,
            tc.tile_pool(name="atp", bufs=2) as atp,
            tc.tile_pool(name="c3w", bufs=3) as c3w,
            tc.tile_pool(name="qkp", bufs=2, space="PSUM") as qkp,
            tc.tile_pool(name="attp", bufs=2, space="PSUM") as attp,
            tc.tile_pool(name="outp_p", bufs=2, space="PSUM") as outp_p,
        ):
            wo_t = w3.tile([P, NE, D], FP16)
            nc.sync.dma_start(out=wo_t, in_=wo_r)
            for tb in range(NTT):
                tbs = slice(tb * TT, (tb + 1) * TT)
                aT = atp.tile([P, NT, TT], FP16, tag="aT")
                for uc in range(NT):
                    qk = qkp.tile([P, TT], F32, tag="qk")
                    nc.tensor.matmul(
                        qk, kT[:, uc * P:(uc + 1) * P], qT[:, tbs],
                        start=True, stop=True,
                    )
                    # a = relu(qk)^2: ACT does relu (psum->fp16),
                    # DVE squares in fp16 (2x mode)
                    rt = c3w.tile([P, TT], FP16, tag="rt")
                    nc.scalar.activation(out=rt, in_=qk, func=AF.Relu)
                    nc.vector.tensor_mul(out=aT[:, uc, :], in0=rt, in1=rt)
                for e in range(NE):
                    at_ps = attp.tile([P, TT], F32, tag="att")
                    for uc in range(NT):
                        nc.tensor.matmul(
                            at_ps, v_t[:, uc, e * P:(e + 1) * P],
                            aT[:, uc, :],
                            start=(uc == 0), stop=(uc == NT - 1),
                        )
                    # g = u * attn, in place over uT (fp16)
                    nc.vector.tensor_mul(
                        out=uT[:, e, tbs], in0=at_ps, in1=uT[:, e, tbs]
                    )
                for tch in range(TT // P):
                    it = tb * (TT // P) + tch
                    tc_ = slice(it * P, (it + 1) * P)
                    osb = c3w.tile([P, D], F32, tag="osb")
                    for (d0, dw) in ((0, 384), (384, 384)):
                        op_ = outp_p.tile([P, 384], F32, tag="mo")
                        for e in range(NE):
                            nc.tensor.matmul(
                                op_, uT[:, e, tc_], wo_t[:, e, d0:d0 + dw],
                                start=(e == 0), stop=(e == NE - 1),
                            )
                        nc.scalar.copy(out=osb[:, d0:d0 + dw], in_=op_)
                    nc.sync.dma_start(out=out_d[tc_, :], in_=osb)

    nc.finalize()
    return nc


def prep_core_inputs(inputs):
    """Host-side slicing: returns the list of 8 per-core input maps."""
    f = np.float32
    x = np.asarray(inputs["x"], f)
    mask = np.asarray(inputs["mask"])
    ln_w = np.asarray(inputs["ln_w"], f)
    ln_b = np.asarray(inputs["ln_b"], f)
    Wz = np.asarray(inputs["Wz"], f)
    bz = np.asarray(inputs["bz"], f)
    Wu = np.asarray(inputs["Wu"], f)
    bu = np.asarray(inputs["bu"], f)
    Wv = np.asarray(inputs["Wv"], f)
    bv = np.asarray(inputs["bv"], f)
    Wo = np.asarray(inputs["Wo"], f)
    gq = np.asarray(inputs["gamma_q"], f)
    bq = np.asarray(inputs["beta_q"], f)
    gk = np.asarray(inputs["gamma_k"], f)
    bk = np.asarray(inputs["beta_k"], f)

    # fold ln_w into the weights, ln_b into the matmul biases
    Wz_e = np.ascontiguousarray(ln_w[:, None] * Wz)
    Wu_e = ln_w[:, None] * Wu
    Wv_e = ln_w[:, None] * Wv
    bz_e = ln_b @ Wz + bz
    bu_e = ln_b @ Wu + bu
    bv_e = ln_b @ Wv + bv

    gqT = np.ascontiguousarray(gq.T)
    bqT = np.ascontiguousarray(bq.T)
    gkT = np.ascontiguousarray(gk.T)
    bkT = np.ascontiguousarray(bk.T)
    # C rows pair with S2b rows: row0 <-> ones (bias), row1 <-> m2 (-colsum)
    Cz = np.stack([bz_e, -Wz_e.sum(0)]).astype(NPFP16)

    in_maps = []
    for c in range(N_CORES):
        b, h = c // 2, c % 2
        cols = slice(h * EH, (h + 1) * EH)
        keep = (~mask[b]).astype(f)  # 1 = attend, 0 = masked-out key
        Wu_h = Wu_e[:, cols]
        Wv_h = Wv_e[:, cols]
        in_maps.append({
            "x": np.ascontiguousarray(x[b]),
            "xT": np.ascontiguousarray(x[b].T.astype(NPFP16)),
            "gqT": gqT,
            "bqT": bqT,
            "gkT": np.ascontiguousarray(gkT * keep[None, :]),
            "bkT": np.ascontiguousarray(bkT * keep[None, :]),
            "Wz": Wz_e.astype(NPFP16),
            "Wu": np.ascontiguousarray(Wu_h.astype(NPFP16)),
            "Wv": np.ascontiguousarray(Wv_h.astype(NPFP16)),
            "Wo": np.ascontiguousarray(Wo[cols, :].astype(NPFP16)),
            "Cz": Cz,
            "Cu": np.ascontiguousarray(
                np.stack([bu_e[cols], -Wu_h.sum(0)]).astype(NPFP16)),
            "Cv": np.ascontiguousarray(
                np.stack([bv_e[cols], -Wv_h.sum(0)]).astype(NPFP16)),
        })
    return in_maps


def combine_outputs(inputs, parts):
    """parts: list of 8 [T, D] partial outputs -> full [B, T, D]."""
    f = np.float32
    x = np.asarray(inputs["x"], f)
    bo = np.asarray(inputs["bo"], f)
    out = np.empty((B, T, D), f)
    for b in range(B):
        out[b] = parts[2 * b] + parts[2 * b + 1] + bo[None, :] + x[b]
    return out


_NC_CACHE = None


def run(inputs, trace=False, **kw):
    global _NC_CACHE
    if _NC_CACHE is None:
        _NC_CACHE = build_module()
    nc = _NC_CACHE
    in_maps = prep_core_inputs(inputs)
    res = run_bass_kernel_spmd(
        nc, in_maps, core_ids=list(range(N_CORES)), trace=trace, **kw
    )
    parts = [r["outp"] for r in res.results]
    return combine_outputs(inputs, parts), res


def kernel(**inputs):
    out, _ = run(inputs)
    return out
